# revision 48
# baseline (speedup 1.0000x reference)
"""Trainium2 Bass kernel for a gated cross-attention block with a dense
per-(b,h) attention bias (B=2, Q=K=2048, C=256, H=8, CH=32).

Sharding: the 16 (batch, 2-head group) units are data-parallel across 8
cores: core i handles batch b = i//4 and heads h0 = 2*(i%4), h0+1.  Linear
weights are column-sliced per head group; the output projection is
tensor-parallel over H*CH, so each core emits a partial [Q, C] output and the
host sums the 4 partials per batch (b_o is injected on exactly one core per
batch by passing zeros to the others).

Per-core dataflow (fp32 accumulate, float32r operands for full PE rate):
  - PE-transpose q_x/kv_x -> xT, project to qT/kT (c-on-partition, with an
    extra ones/mask row so Q@K^T + mask comes out of one contraction),
    gT (sigmoid with per-partition b_g bias), v (natural, with a ones column
    appended so the softmax denominator falls out of the AV matmul).
  - scores^T tiles [128 k, 512 q]: PE-transposes of the bias tile land in
    PSUM, then the QK^T matmul accumulates on top (start=False) -- the bias
    add costs zero vector-engine time.
  - exp on ScalarE reads PSUM, writes attn^T to SBUF (rounded to f32r).
  - AV matmul contracts k per 128-chunk with v_aug stationary; row 32 of the
    output is the softmax denominator.  Reciprocal + ones-matmul broadcast +
    two vector multiplies produce the gated, normalized og^T.
  - Final matmul og^T.T @ w_o (+ b_o via a ones-vector matmul) -> partial out.
"""

import math

import numpy as np

B, Q, K, C, H, CH = 2, 2048, 2048, 256, 8, 32
N_CORES = 8
HPC = 2            # heads per core
GROUPS = H // HPC  # head groups per batch = 4

_cache = {}


def _build_nc(q=Q, k=K, tmode="f32r"):
    """Emit the per-core Bass program. q/k overridable for small-scale sim.

    tmode: dtype scheme for PE transposes. "bf16" streams a bf16 identity
    (1 cyc/col) against f32r-bitcast data; "f32" is the conservative
    2 cyc/col fp32 path.
    """
    import concourse.bacc as bacc
    import concourse.mybir as mybir
    import concourse.tile as tile
    from concourse.masks import make_identity

    f32 = mybir.dt.float32
    f32r = mybir.dt.float32r
    bf16 = mybir.dt.bfloat16
    AF = mybir.ActivationFunctionType

    id_dt = f32r if tmode == "f32r" else (bf16 if tmode == "bf16" else f32)
    # dtype for DMA-loaded tiles that feed PE transposes: declaring them f32r
    # at the DMA (bitcast both sides) makes the DMACopy the f32r producer
    tr_dt = f32r if tmode == "f32r" else f32

    def tr_in(dram_ap):
        return dram_ap.bitcast(f32r) if tmode == "f32r" else dram_ap

    def tp_args(data_ap, out_ap):
        if tmode in ("bf16", "f32r"):
            return data_ap.bitcast(f32r), out_ap.bitcast(f32r)
        return data_ap, out_ap

    nqc = q // 512        # 512-wide q chunks
    nkc = k // 128        # 128-wide k chunks
    nqt = q // 128        # 128-row q tiles
    ncc = C // 128        # 128-row c chunks (2)

    nc = bacc.Bacc(
        "TRN2", target_bir_lowering=False, debug=False, num_devices=N_CORES
    )

    qx_d = nc.dram_tensor("qx", [q, C], f32, kind="ExternalInput").ap()
    kvx_d = nc.dram_tensor("kvx", [k, C], f32, kind="ExternalInput").ap()
    mask_d = nc.dram_tensor("mask", [1, k], f32, kind="ExternalInput").ap()
    tri_d = nc.dram_tensor("tri", [HPC, q, k], f32, kind="ExternalInput").ap()
    wq_d = nc.dram_tensor("wq", [C, HPC * CH], f32, kind="ExternalInput").ap()
    wk_d = nc.dram_tensor("wk", [C, HPC * CH], f32, kind="ExternalInput").ap()
    wv_d = nc.dram_tensor("wv", [C, HPC * CH], f32, kind="ExternalInput").ap()
    wg_d = nc.dram_tensor("wg", [C, HPC * CH], f32, kind="ExternalInput").ap()
    bg_d = nc.dram_tensor("bg", [HPC * CH, 1], f32, kind="ExternalInput").ap()
    wo_d = nc.dram_tensor("wo", [HPC * CH, C], f32, kind="ExternalInput").ap()
    bo_d = nc.dram_tensor("bo", [1, C], f32, kind="ExternalInput").ap()
    out_d = nc.dram_tensor("out_p", [q, C], f32, kind="ExternalOutput").ap()

    inv_sqrt_ch = 1.0 / math.sqrt(CH)

    with tile.TileContext(nc) as tc:
        with (
            tc.tile_pool(name="const", bufs=1) as const,
            tc.tile_pool(name="persist", bufs=1) as persist,
        ):
            identity_f32 = const.tile([128, 128], f32)
            make_identity(nc, identity_f32)
            if id_dt == f32:
                identity = identity_f32
            else:
                identity = const.tile([128, 128], id_dt)
                nc.vector.tensor_copy(identity, identity_f32)
            wq_sb = const.tile([128, ncc * HPC * CH], f32r)
            wk_sb = const.tile([128, ncc * HPC * CH], f32r)
            wv_sb = const.tile([128, ncc * HPC * CH], f32r)
            wg_sb = const.tile([128, ncc * HPC * CH], f32r)
            wo_sb = const.tile([HPC * CH, C], f32r)
            bo_sb = const.tile([1, C], f32r)
            bg_col = const.tile([HPC * CH, 1], f32)
            ones_1x32 = const.tile([1, 32], f32r)
            bo_bc = const.tile([128, C], f32)
            # persistent activations: qT/kT hold per-head 64-row slots
            # (rows h*64 .. h*64+31 = channels, row h*64+32 = ones / mask)
            qT_sb = persist.tile([128, q], f32r)
            kT_sb = persist.tile([128, k], f32r)
            gT_sb = persist.tile([HPC * CH, q], f32)
            # per-chunk layout [v (32) | ones (1) | zeros (31)] -- padded to 64
            # so the packed AV output exactly fills a 64-row PSUM col group
            VW = 64
            v_aug = [persist.tile([128, nkc * VW], f32r, name=f"vaug{h}")
                     for h in range(HPC)]
            ogT_sb = persist.tile([HPC * CH, q], f32r)

            # ---------------- phase 0: transposes + projections ----------
            with (
                tc.tile_pool(name="ph0", bufs=1) as ph0,
                tc.tile_pool(name="ph0ps", bufs=2, space="PSUM") as ph0ps,
            ):
                qx_sb = ph0.tile([128, nqt * C], tr_dt)
                kvx_sb = ph0.tile([128, (k // 128) * C], tr_dt)
                qxT_sb = ph0.tile([128, ncc * q], f32r)
                kvxT_sb = ph0.tile([128, ncc * k], f32r)
                # x loads first (they gate everything); chunked so the first
                # transposes can start early
                for x_sb, x_d, nt in ((qx_sb, qx_d, nqt), (kvx_sb, kvx_d, k // 128)):
                    nch = 4
                    step = nt // nch
                    for ch in range(nch):
                        nc.sync.dma_start(
                            out=x_sb.rearrange("p (n c) -> p n c", c=C)[
                                :, ch * step : (ch + 1) * step, :
                            ],
                            in_=tr_in(x_d).rearrange("(n p) c -> p n c", p=128)[
                                :, ch * step : (ch + 1) * step, :
                            ],
                        )

                ones_st = ph0.tile([1, max(q, 2048)], f32)
                mask_st = ph0.tile([1, k], f32)
                ones_1x128 = ph0.tile([1, 128], f32r)
                # fp32 staging for DMA'd weights; rounded copies feed matmuls
                wq_st = ph0.tile([128, ncc * HPC * CH], f32)
                wk_st = ph0.tile([128, ncc * HPC * CH], f32)
                wv_st = ph0.tile([128, ncc * HPC * CH], f32)
                wg_st = ph0.tile([128, ncc * HPC * CH], f32)
                wo_st = ph0.tile([HPC * CH, C], f32)
                bo_st = ph0.tile([1, C], f32)
                for w_d, w_st in ((wq_d, wq_st), (wk_d, wk_st), (wv_d, wv_st), (wg_d, wg_st)):
                    nc.sync.dma_start(
                        out=w_st.rearrange("p (t m) -> p t m", t=ncc),
                        in_=w_d.rearrange("(t p) m -> p t m", p=128),
                    )
                nc.sync.dma_start(out=wo_st, in_=wo_d)
                nc.sync.dma_start(out=bo_st, in_=bo_d)
                nc.sync.dma_start(out=bg_col, in_=bg_d)
                nc.sync.dma_start(out=mask_st, in_=mask_d)
                for st, sb in ((wq_st, wq_sb), (wk_st, wk_sb), (wv_st, wv_sb),
                               (wg_st, wg_sb), (wo_st, wo_sb), (bo_st, bo_sb)):
                    nc.vector.tensor_copy(sb, st)
                nc.vector.memset(ones_st, 1.0)
                nc.vector.tensor_copy(ones_1x32, ones_st[:, :32])
                nc.vector.tensor_copy(ones_1x128, ones_st[:, :128])
                nc.vector.tensor_copy(kT_sb[32:33, :], mask_st)
                nc.vector.tensor_copy(kT_sb[96:97, :], mask_st)
                nc.vector.tensor_copy(qT_sb[32:33, :], ones_st[:, :q])
                nc.vector.tensor_copy(qT_sb[96:97, :], ones_st[:, :q])
                # b_o broadcast across partitions, used in the output phase
                pbo = ph0ps.tile([128, C], f32, tag="proj2")
                nc.tensor.matmul(pbo, ones_1x128, bo_sb)
                nc.vector.tensor_copy(bo_bc, pbo)

                # x transposes: 4 per PSUM tile, copies alternate DVE/ACT
                nxt = 0
                for x_sb, xT_sb, nt in (
                    (qx_sb, qxT_sb, nqt),
                    (kvx_sb, kvxT_sb, k // 128),
                ):
                    for n in range(nt):
                        tp = ph0ps.tile([128, 512], f32, tag="t0",
                                        name=f"tp{nxt}")
                        for cc in range(ncc):
                            xin, xout = tp_args(
                                x_sb[:, n * C + cc * 128 : n * C + cc * 128 + 128],
                                tp[:, cc * 128 : cc * 128 + 128],
                            )
                            nc.tensor.matmul(
                                xout,
                                xin,
                                identity,
                                is_transpose=True,
                                start=(cc == 0),
                                stop=(cc == ncc - 1),
                            )
                        # scatter the ncc c-chunks to their xT positions
                        for cc in range(ncc):
                            dst = xT_sb[:, cc * (nt * 128) + n * 128 :
                                        cc * (nt * 128) + n * 128 + 128]
                            src = tp[:, cc * 128 : cc * 128 + 128]
                            if nxt % 2 == 0:
                                nc.vector.tensor_copy(dst, src)
                            else:
                                nc.scalar.copy(dst, src)
                        nxt += 1

                # qT / kT projections, one head (32 rows) at a time
                for h in range(HPC):
                    for qn in range(nqc):
                        pq = ph0ps.tile([32, 512], f32, tag="proj")
                        for cc in range(ncc):
                            nc.tensor.matmul(
                                pq,
                                wq_sb[:, cc * 64 + h * 32 : cc * 64 + h * 32 + 32],
                                qxT_sb[:, cc * q + qn * 512 : cc * q + qn * 512 + 512],
                                start=(cc == 0),
                                stop=(cc == ncc - 1),
                            )
                        nc.scalar.mul(
                            qT_sb[h * 64 : h * 64 + 32, qn * 512 : qn * 512 + 512],
                            pq,
                            inv_sqrt_ch,
                        )
                    for kn in range(k // 512):
                        pk = ph0ps.tile([32, 512], f32, tag="proj")
                        for cc in range(ncc):
                            nc.tensor.matmul(
                                pk,
                                wk_sb[:, cc * 64 + h * 32 : cc * 64 + h * 32 + 32],
                                kvxT_sb[:, cc * k + kn * 512 : cc * k + kn * 512 + 512],
                                start=(cc == 0),
                                stop=(cc == ncc - 1),
                            )
                        nc.vector.tensor_copy(
                            kT_sb[h * 64 : h * 64 + 32, kn * 512 : kn * 512 + 512],
                            pk,
                        )

                # gT (both heads at once), sigmoid with per-partition b_g
                for qn in range(nqc):
                    pg = ph0ps.tile([HPC * CH, 512], f32, tag="proj2")
                    for cc in range(ncc):
                        nc.tensor.matmul(
                            pg,
                            wg_sb[:, cc * 64 : cc * 64 + 64],
                            qxT_sb[:, cc * q + qn * 512 : cc * q + qn * 512 + 512],
                            start=(cc == 0),
                            stop=(cc == ncc - 1),
                        )
                    nc.scalar.activation(
                        gT_sb[:, qn * 512 : qn * 512 + 512],
                        pg,
                        AF.Sigmoid,
                        bias=bg_col,
                    )

                # v natural [k, 64] -> per-head augmented [128, nkc*(CH+1)]
                for kn in range(nkc):
                    pv = ph0ps.tile([128, HPC * CH], f32, tag="projv")
                    for cc in range(ncc):
                        nc.tensor.matmul(
                            pv,
                            kvxT_sb[:, cc * k + kn * 128 : cc * k + kn * 128 + 128],
                            wv_sb[:, cc * 64 : cc * 64 + 64],
                            start=(cc == 0),
                            stop=(cc == ncc - 1),
                        )
                    for h in range(HPC):
                        nc.vector.tensor_copy(
                            v_aug[h][:, kn * VW : kn * VW + CH],
                            pv[:, h * CH : h * CH + CH],
                        )
                # fill [ones | zeros] tail columns of v_aug via one fp32
                # staging tile (memset cannot produce f32r directly)
                vfill = ph0.tile([128, nkc * (VW - CH)], f32)
                vfill3 = vfill.rearrange("p (n c) -> p n c", c=VW - CH)
                nc.vector.memset(vfill, 0.0)
                nc.vector.memset(vfill3[:, :, 0], 1.0)
                for h in range(HPC):
                    va3 = v_aug[h].rearrange("p (n c) -> p n c", c=VW)
                    nc.vector.tensor_copy(va3[:, :, CH:VW], vfill3)

            # ---------------- main loop ---------------------------------
            # Heads are interleaved per (qc, kc2) so the two heads' QK and AV
            # matmuls sit adjacent in the PE stream with different
            # tile_positions (row groups 0/64 for QK, col groups 0/64 for AV)
            # and execute concurrently in the array's quadrants.
            khalf = nkc // 2
            with (
                tc.tile_pool(name="bias", bufs=18) as biasp,
                tc.tile_pool(name="attn", bufs=1) as attnp,
                tc.tile_pool(name="small", bufs=2) as smallp,
                tc.tile_pool(name="mainps", bufs=1, space="PSUM") as mps,
            ):
                for qc in range(nqc):
                    bts = {}
                    for kh in range(2):
                        for h in range(HPC):
                            for j in range(4):
                                bt = biasp.tile(
                                    [128, k // 2], tr_dt, tag="bias",
                                    name=f"bt{qc}_{kh}_{h}_{j}",
                                )
                                nc.sync.dma_start(
                                    out=bt,
                                    in_=tr_in(tri_d)[
                                        h,
                                        qc * 512 + j * 128 : qc * 512 + j * 128 + 128,
                                        kh * (k // 2) : (kh + 1) * (k // 2),
                                    ],
                                )
                                bts[(kh, h, j)] = bt
                    attnT = [
                        attnp.tile([128, nkc * 512], f32r, tag=f"attnT{h}",
                                   name=f"attnT{h}_{qc}")
                        for h in range(HPC)
                    ]
                    for kc2 in range(khalf):
                        Ss = []
                        for h in range(HPC):
                            S2 = mps.tile([128, 1024], f32, tag="S", bufs=3,
                                          name=f"S{h}_{qc}_{kc2}")
                            Ss.append(S2)
                            for t in range(2):
                                kc = kc2 * 2 + t
                                kh, kcol = divmod(kc, khalf)
                                for j in range(4):
                                    bin_, bout = tp_args(
                                        bts[(kh, h, j)][:, kcol * 128 : kcol * 128 + 128],
                                        S2[:, t * 512 + j * 128 : t * 512 + j * 128 + 128],
                                    )
                                    nc.tensor.matmul(
                                        bout,
                                        bin_,
                                        identity,
                                        is_transpose=True,
                                        start=(j == 0),
                                        stop=False,
                                    )
                        for t in range(2):
                            kc = kc2 * 2 + t
                            for h in range(HPC):
                                hb = h * 64
                                nc.tensor.matmul(
                                    Ss[h][:, t * 512 : t * 512 + 512],
                                    kT_sb[hb : hb + 33, kc * 128 : kc * 128 + 128],
                                    qT_sb[hb : hb + 33, qc * 512 : qc * 512 + 512],
                                    start=False,
                                    stop=True,
                                )
                        for h in range(HPC):
                            nc.scalar.activation(
                                attnT[h][:, kc2 * 1024 : kc2 * 1024 + 1024],
                                Ss[h],
                                AF.Exp,
                            )
                    # AV with fused denominator (ones column of v_aug);
                    # per-head PSUM tiles (walrus requires matmul dst at
                    # partition base 0)
                    o_aug = [
                        mps.tile([64, 512], f32, tag=f"av{h}", bufs=1,
                                 name=f"oaug{qc}_{h}")
                        for h in range(HPC)
                    ]
                    for kc in range(nkc):
                        for h in range(HPC):
                            nc.tensor.matmul(
                                o_aug[h][:, :],
                                v_aug[h][:, kc * VW : (kc + 1) * VW],
                                attnT[h][:, kc * 512 : kc * 512 + 512],
                                start=(kc == 0),
                                stop=(kc == nkc - 1),
                            )
                    # SBUF-SBUF tensor_tensor inputs must share a start
                    # partition, so the broadcast / gating tiles are sliced at
                    # each head's base (h*32) to line up with gT_sb / ogT_sb
                    rbc_full = smallp.tile([HPC * CH, 512], f32, tag="rbc",
                                           bufs=2, name=f"rbc{qc}")
                    gtmp_full = smallp.tile([HPC * CH, 512], f32, tag="gtmp",
                                            bufs=2, name=f"gtmp{qc}")
                    for h in range(HPC):
                        recip_f = smallp.tile([1, 512], f32, tag=f"recipf{h}",
                                              bufs=1, name=f"recf{qc}_{h}")
                        nc.vector.reciprocal(recip_f, o_aug[h][CH : CH + 1, :])
                        # broadcast 1/sum across 32 partitions on the (idle)
                        # GPSIMD engine instead of spending PSUM + PE on it.
                        # partition_broadcast only writes correctly at
                        # partition base 0, so h1 goes through a bounce tile.
                        r_bc = rbc_full[h * CH : h * CH + CH, :]
                        if h == 0:
                            nc.gpsimd.partition_broadcast(r_bc, recip_f)
                        else:
                            bc_tmp = smallp.tile([CH, 512], f32, tag="bctmp",
                                                 bufs=1, name=f"bct{qc}_{h}")
                            nc.gpsimd.partition_broadcast(bc_tmp, recip_f)
                            nc.vector.tensor_copy(r_bc, bc_tmp)
                        gtmp = gtmp_full[h * CH : h * CH + CH, :]
                        nc.vector.tensor_mul(
                            gtmp,
                            gT_sb[h * CH : h * CH + CH, qc * 512 : qc * 512 + 512],
                            r_bc,
                        )
                        nc.vector.tensor_mul(
                            ogT_sb[h * CH : h * CH + CH, qc * 512 : qc * 512 + 512],
                            gtmp,
                            o_aug[h][0:CH, :],
                        )
                    # output projection for this q block, pipelined with the
                    # next block's scores
                    ob = smallp.tile([128, 4 * C], f32, tag="ob",
                                     name=f"ob{qc}")
                    for s in range(4):
                        qs = qc * 4 + s
                        op = mps.tile([128, C], f32, tag="av1", bufs=1,
                                      name=f"op{qc}_{s}")
                        nc.tensor.matmul(
                            op,
                            ogT_sb[:, qs * 128 : qs * 128 + 128],
                            wo_sb,
                        )
                        nc.vector.tensor_add(ob[:, s * C : s * C + C], op, bo_bc)
                    nc.sync.dma_start(
                        out=out_d[qc * 512 : qc * 512 + 512, :].rearrange(
                            "(n p) c -> p n c", p=128
                        ),
                        in_=ob.rearrange("p (n c) -> p n c", c=C),
                    )
    nc.compile()
    return nc


def _shard_inputs(q_x, kv_x, mask_bias, triangle_bias, w_q, w_k, w_v, w_g,
                  b_g, w_o, b_o):
    """Build the 8 per-core input maps."""
    in_maps = []
    for core in range(N_CORES):
        b = core // GROUPS
        g = core % GROUPS
        h0 = g * HPC
        cs = slice(h0 * CH, (h0 + HPC) * CH)
        bo = b_o if g == 0 else np.zeros_like(b_o)
        in_maps.append({
            "qx": np.ascontiguousarray(q_x[b]),
            "kvx": np.ascontiguousarray(kv_x[b]),
            "mask": np.ascontiguousarray(mask_bias[b, 0, 0]).reshape(1, K),
            "tri": np.ascontiguousarray(triangle_bias[b, h0 : h0 + HPC]),
            "wq": np.ascontiguousarray(w_q[:, cs]),
            "wk": np.ascontiguousarray(w_k[:, cs]),
            "wv": np.ascontiguousarray(w_v[:, cs]),
            "wg": np.ascontiguousarray(w_g[:, cs]),
            "bg": np.ascontiguousarray(b_g[cs]).reshape(HPC * CH, 1),
            "wo": np.ascontiguousarray(w_o[cs, :]),
            "bo": np.ascontiguousarray(bo).reshape(1, C),
        })
    return in_maps


def kernel(**inputs):
    from concourse import bass_utils

    inputs = {k_: np.asarray(v, dtype=np.float32) for k_, v in inputs.items()}
    if "nc" not in _cache:
        _cache["nc"] = _build_nc()
    nc = _cache["nc"]

    in_maps = _shard_inputs(**inputs)
    res = bass_utils.run_bass_kernel_spmd(nc, in_maps, core_ids=list(range(N_CORES)))

    out = np.zeros((B, Q, C), np.float32)
    for core in range(N_CORES):
        out[core // GROUPS] += res.results[core]["out_p"]
    return out
